# revision 1
# baseline (speedup 1.0000x reference)
"""Multi-head attention (B=2, S=2048, D=768, H=12) on 8 TRN2 NeuronCores.

Sharding: core c -> batch b = c//4, head-group g = c%4 (3 heads of 64 each).
Each core computes q/k/v projections for its 3 heads, masked softmax
attention (transposed-energy formulation, denominator via a ones column
appended to v), and a partial output projection against its 192 columns of
Wo. Host sums the 4 partial outputs per batch element.

Device layout notes:
  - Projections produce qT/kT/vT (d on partitions) via W-stationary matmuls.
  - E^T[k,q] accumulates with K=64; heads 0/1 are packed in partitions
    0-63/64-127 of one tile so their matmuls run on distinct PE row groups.
  - P^T = exp(scale*E^T) * maskT(0/1, bf16) ; out_un^T = [v|1]^T-style
    matmul (ones column -> denominator row 64 of the PSUM tile).
  - Normalisation happens on the small out_un^T (65 x 512) tiles, not on P.
  - All f32 matmuls are issued as float32r (full-rate for free dim >= 256).
"""

import os
import sys

sys.path.insert(0, "/opt/trn_rl_repo")

from contextlib import ExitStack

import ml_dtypes
import numpy as np

import concourse.bass as bass
import concourse.mybir as mybir
import concourse.tile as tile
from concourse import bacc
from concourse.bass import ds
from concourse.bass_utils import run_bass_kernel_spmd
from concourse.masks import make_identity

F32 = mybir.dt.float32
F32R = mybir.dt.float32r
BF16 = mybir.dt.bfloat16

SEQ = 2048
D = 768
HD = 64
GD = 192          # head-group width = 3 heads * 64
QB = 512          # q-block (free dim of E^T matmuls)
NQB = SEQ // QB   # 4
KT = SEQ // 128   # 16 k-tiles
SCALE = float(1.0 / np.sqrt(np.float32(D)))

_CACHE = {}


def _install_profile_hook():
    """The image's antenv lacks axon_hooks; synthesize it so
    run_bass_kernel_spmd(trace=True) can reach the NTFF profiler in
    libaxon_pjrt.so (same ctypes shim trn_agent_boot uses)."""
    import types

    if "antenv.axon_hooks" in sys.modules:
        return
    sys.path.insert(0, "/root/.axon_site")
    try:
        from trn_agent_boot.trn_boot import _ntff_profile_via_ctypes
        hook = _ntff_profile_via_ctypes("/opt/axon/libaxon_pjrt.so")
    except Exception:
        hook = None
    import concourse.bass_utils as _bu

    _bu.upload_artifacts = lambda tmpdir: tmpdir  # no artifact bucket here
    mod = types.ModuleType("antenv.axon_hooks")
    mod.get_axon_ntff_profile_hook = lambda: hook
    mod.set_axon_ntff_profile_hook = lambda h: None
    sys.modules["antenv.axon_hooks"] = mod



def _build():
    nc = bacc.Bacc(None)

    xqT = nc.declare_dram_parameter("xqT", [D, SEQ], BF16, isOutput=False)
    xkT = nc.declare_dram_parameter("xkT", [D, SEQ], BF16, isOutput=False)
    xvT = nc.declare_dram_parameter("xvT", [D, SEQ], BF16, isOutput=False)
    wqT = nc.declare_dram_parameter("wqT", [D, GD], BF16, isOutput=False)
    wkT = nc.declare_dram_parameter("wkT", [D, GD], BF16, isOutput=False)
    wvT = nc.declare_dram_parameter("wvT", [D, GD], BF16, isOutput=False)
    woT = nc.declare_dram_parameter("woT", [GD, D], BF16, isOutput=False)
    maskT = nc.declare_dram_parameter("maskT", [SEQ, SEQ], BF16, isOutput=False)
    out = nc.declare_dram_parameter("out", [SEQ, D], F32, isOutput=True)

    with tile.TileContext(nc) as tc, ExitStack() as ctx:
        Exp = mybir.ActivationFunctionType.Exp

        cpool = ctx.enter_context(tc.tile_pool(name="const", bufs=1))
        ident = cpool.tile([128, 128], BF16)
        make_identity(nc, ident[:])

        # ---- persistent activations --------------------------------------
        pp = ctx.enter_context(tc.tile_pool(name="persist", bufs=1))
        qA = pp.tile([128, SEQ], BF16, tag="qA")   # heads 0 (p0-63) and 1 (p64-127)
        qB = pp.tile([64, SEQ], BF16, tag="qB")    # head 2
        kA = pp.tile([128, SEQ], BF16, tag="kA")
        kB = pp.tile([64, SEQ], BF16, tag="kB")
        vaug = [pp.tile([128, KT, HD + 1], BF16, tag=f"vaug{h}", name=f"vaug{h}") for h in range(3)]
        onorm = [pp.tile([64, SEQ], BF16, tag=f"onorm{h}", name=f"onorm{h}") for h in range(3)]
        wo_sb = [pp.tile([64, D], BF16, tag=f"wo{h}", name=f"wo{h}") for h in range(3)]

        for h in range(3):
            nc.sync.dma_start(wo_sb[h][:], woT[ds(h * 64, 64), :])
            # ones column for the softmax denominator
            nc.vector.memset(vaug[h][:, :, HD : HD + 1], 1.0)

        # ---- phase 1: projections + v transpose --------------------------
        wp = ctx.enter_context(tc.tile_pool(name="wp", bufs=1))
        xp = ctx.enter_context(tc.tile_pool(name="xp", bufs=9))
        vtp = ctx.enter_context(tc.tile_pool(name="vt", bufs=1))
        with tc.tile_pool(name="pj_ps", bufs=2, space="PSUM") as pj_ps, \
             tc.tile_pool(name="tr_ps", bufs=2, space="PSUM") as tr_ps:

            w_sb = {}
            for name, wT in (("q", wqT), ("k", wkT), ("v", wvT)):
                w_sb[name] = wp.tile([128, 6, GD], BF16, tag=f"w{name}", name=f"w_{name}")
                nc.sync.dma_start(
                    w_sb[name][:], wT.rearrange("(ko ki) d -> ki ko d", ki=128)
                )

            vtA = vtp.tile([128, SEQ], BF16, tag="vtA")
            vtB = vtp.tile([64, SEQ], BF16, tag="vtB")

            dests = {"q": (qA, qB), "k": (kA, kB), "v": (vtA, vtB)}
            for name, xT in (("q", xqT), ("k", xkT), ("v", xvT)):
                for nb2 in range(2):  # halve DMA count: 1024-wide x tiles
                    xk = []
                    for k in range(6):
                        xt = xp.tile(
                            [128, 1024], BF16, tag="x", name=f"x_{name}_{nb2}_{k}"
                        )
                        nc.gpsimd.dma_start(
                            xt[:], xT[ds(k * 128, 128), ds(nb2 * 1024, 1024)]
                        )
                        xk.append(xt)
                    for half in range(2):
                        n = nb2 * 2 + half
                        for mt in range(2):
                            mw = 128 if mt == 0 else 64
                            ps = pj_ps.tile([128, QB], F32, tag="pjps")
                            for k in range(6):
                                nc.tensor.matmul(
                                    ps[0:mw, :],
                                    lhsT=w_sb[name][:, k, ds(mt * 128, mw)],
                                    rhs=xk[k][:, ds(half * QB, QB)],
                                    start=(k == 0),
                                    stop=(k == 5),
                                )
                            dst = dests[name][0] if mt == 0 else dests[name][1]
                            if name == "v":
                                nc.vector.tensor_copy(
                                    dst[0:mw, ds(n * QB, QB)], ps[0:mw, :]
                                )
                            else:
                                nc.scalar.copy(
                                    dst[0:mw, ds(n * QB, QB)], ps[0:mw, :]
                                )

            # transpose vT -> v_aug (bf16), per 128-seq block
            for s in range(KT):
                ptA = tr_ps.tile([128, 128], BF16, tag="ptA")
                nc.tensor.transpose(ptA[:], vtA[:, ds(s * 128, 128)], ident[:])
                nc.vector.tensor_copy(vaug[0][:, s, 0:HD], ptA[:, 0:64])
                nc.vector.tensor_copy(vaug[1][:, s, 0:HD], ptA[:, 64:128])
                ptB = tr_ps.tile([128, 64], BF16, tag="ptB")
                nc.tensor.transpose(
                    ptB[:], vtB[0:64, ds(s * 128, 128)], ident[0:64, 0:64]
                )
                nc.vector.tensor_copy(vaug[2][:, s, 0:HD], ptB[:, 0:64])

        # ---- phase 2: attention ------------------------------------------
        mp = ctx.enter_context(tc.tile_pool(name="mp", bufs=2))
        pp2 = ctx.enter_context(tc.tile_pool(name="pp2", bufs=3))
        rp = ctx.enter_context(tc.tile_pool(name="rp", bufs=2))
        with tc.tile_pool(name="e_ps", bufs=2, space="PSUM") as e_ps, \
             tc.tile_pool(name="ou_ps", bufs=2, space="PSUM") as ou_ps:

            q_of = (qA, qA, qB)
            k_of = (kA, kA, kB)
            pbase = (0, 64, 0)

            for n in range(NQB):
                mask_t = mp.tile([128, KT, QB], BF16, tag="mask")
                for j in range(KT):
                    nc.gpsimd.dma_start(
                        mask_t[:, j, :],
                        maskT[ds(j * 128, 128), ds(n * QB, QB)],
                    )
                P = [pp2.tile([128, KT, QB], BF16, tag="P", name=f"P{n}_{i}") for i in range(3)]
                for grp in range(KT // 2):
                    for h in range(3):
                        e = e_ps.tile([128, 2, QB], F32, tag="e")
                        for mm in range(2):
                            m = grp * 2 + mm
                            p0 = pbase[h]
                            nc.tensor.matmul(
                                e[:, mm, :],
                                lhsT=k_of[h][
                                    p0 : p0 + 64, ds(m * 128, 128)
                                ],
                                rhs=q_of[h][p0 : p0 + 64, ds(n * QB, QB)],
                                start=True,
                                stop=True,
                            )
                        sl = ds(grp * 2, 2)
                        nc.scalar.activation(
                            P[h][:, sl, :], e[:, :, :], Exp, scale=SCALE
                        )
                        nc.vector.tensor_mul(
                            P[h][:, sl, :], P[h][:, sl, :], mask_t[:, sl, :]
                        )
                for h in range(3):
                    ou = ou_ps.tile([HD + 1, QB], F32, tag="ou")
                    for m in range(KT):
                        nc.tensor.matmul(
                            ou[:],
                            lhsT=vaug[h][:, m, :],
                            rhs=P[h][:, m, :],
                            start=(m == 0),
                            stop=(m == KT - 1),
                        )
                    r1 = rp.tile([1, QB], F32, tag="r1")
                    nc.vector.reciprocal(r1[:], ou[HD : HD + 1, :])
                    rb = rp.tile([64, QB], F32, tag="rb")
                    nc.gpsimd.partition_broadcast(rb[:], r1[:])
                    nc.vector.tensor_mul(
                        onorm[h][:, ds(n * QB, QB)], ou[0:HD, :], rb[:]
                    )

        # ---- phase 3: output projection ----------------------------------
        op = ctx.enter_context(tc.tile_pool(name="op", bufs=3))
        with tc.tile_pool(name="fp", bufs=2, space="PSUM") as f_ps:
            for mq in range(SEQ // 128):
                fp = f_ps.tile([128, D], F32, tag="f")
                for n0, nw in ((0, 512), (512, 256)):
                    for h in range(3):
                        nc.tensor.matmul(
                            fp[:, ds(n0, nw)],
                            lhsT=onorm[h][:, ds(mq * 128, 128)],
                            rhs=wo_sb[h][:, ds(n0, nw)],
                            start=(h == 0),
                            stop=(h == 2),
                        )
                o_sb = op.tile([128, D], F32, tag="o")
                nc.scalar.copy(o_sb[:], fp[:])
                nc.gpsimd.dma_start(out[ds(mq * 128, 128), :], o_sb[:])

    nc.compile()
    return nc


def kernel(Q, K, V, mask, Wq, Wk, Wv, Wo):
    if "nc" not in _CACHE:
        _CACHE["nc"] = _build()
    nc = _CACHE["nc"]

    maskT_bf = np.ascontiguousarray(
        (mask[0, 0].T != 0).astype(ml_dtypes.bfloat16)
    )
    in_maps = []
    for c in range(8):
        b, g = c // 4, c % 4
        sl = slice(g * GD, (g + 1) * GD)
        in_maps.append(
            {
                "xqT": np.ascontiguousarray(Q[b].T.astype(ml_dtypes.bfloat16)),
                "xkT": np.ascontiguousarray(K[b].T.astype(ml_dtypes.bfloat16)),
                "xvT": np.ascontiguousarray(V[b].T.astype(ml_dtypes.bfloat16)),
                "wqT": np.ascontiguousarray(Wq[sl, :].T.astype(ml_dtypes.bfloat16)),
                "wkT": np.ascontiguousarray(Wk[sl, :].T.astype(ml_dtypes.bfloat16)),
                "wvT": np.ascontiguousarray(Wv[sl, :].T.astype(ml_dtypes.bfloat16)),
                "woT": np.ascontiguousarray(Wo[:, sl].T.astype(ml_dtypes.bfloat16)),
                "maskT": maskT_bf,
            }
        )

    _install_profile_hook()
    res = run_bass_kernel_spmd(
        nc,
        in_maps,
        core_ids=list(range(8)),
        trace=bool(int(os.environ.get("KERNEL_PROFILE", "0"))),
    )
    _CACHE["last_exec_ns"] = res.exec_time_ns

    out = np.zeros((2, SEQ, D), dtype=np.float32)
    for c in range(8):
        out[c // 4] += res.results[c]["out"]
    return out



# revision 22
# speedup vs baseline: 1.2227x; 1.2227x over previous
"""Multi-head attention (B=2, S=2048, D=768, H=12) on 8 TRN2 NeuronCores.

Sharding: core c -> batch b = c//4, head-group g = c%4 (3 heads of 64 each).
Each core computes q/k/v projections for its 3 heads, masked softmax
attention, and a partial output projection against its 192 columns of Wo.
Host sums the 4 partial outputs per batch element (fp16 partials, fp32 sum).

Perf notes (v2):
  - fp16 everywhere on-device (same cost as bf16, more mantissa).
  - PE is HAM-clock-gated (1.2 GHz cold, 2.4 GHz after ~3.4us of sustained
    work): warm-up matmuls run during the initial DMA wait and the whole
    kernel is emitted as one gap-free tensor stream.
  - v is projected directly into [seq, d] layout (x-stationary matmuls) so
    no PE transposes are needed.
  - Attention is a 1-step-lagged pipeline over (n-block, k-group) steps:
    energy matmuls for group g run while exp/mask of g-1 and attV of g-1
    run on scalar/vector, keeping all engines busy.
  - Softmax denominators: ones-column in the v tiles -> row 64 of the attV
    PSUM; reciprocal via the fast custom-DVE approx on [1,512] (the plain
    reciprocal costs 3.3us), broadcast on gpsimd.
  - Output projection packs heads 0+1 into one K=128 matmul (onA) plus a
    K=64 accumulate (onB), interleaved one chain per pipeline step.
"""

import os
import sys

sys.path.insert(0, "/opt/trn_rl_repo")

from contextlib import ExitStack

import numpy as np

import concourse.bass as bass
import concourse.mybir as mybir
import concourse.tile as tile
from concourse import bacc
from concourse.bass import ds
from concourse.bass_utils import run_bass_kernel_spmd

F32 = mybir.dt.float32
F16 = mybir.dt.float16

SEQ = 2048
D = 768
HD = 64
GD = 192          # head-group width = 3 heads * 64
QB = 512          # q-block (free dim of E^T matmuls)
NQB = SEQ // QB   # 4
KT = SEQ // 128   # 16 k-tiles
NG = KT // 2      # 8 k-groups of 2 tiles per q-block
SCALE = float(1.0 / np.sqrt(np.float32(D)))

_CACHE = {}


def _install_profile_hook():
    """The image's antenv lacks axon_hooks; synthesize it so
    run_bass_kernel_spmd(trace=True) can reach the NTFF profiler in
    libaxon_pjrt.so (same ctypes shim trn_agent_boot uses)."""
    import types

    if "antenv.axon_hooks" in sys.modules:
        return
    sys.path.insert(0, "/root/.axon_site")
    try:
        from trn_agent_boot.trn_boot import _ntff_profile_via_ctypes
        hook = _ntff_profile_via_ctypes("/opt/axon/libaxon_pjrt.so")
    except Exception:
        hook = None
    import concourse.bass_utils as _bu

    _bu.upload_artifacts = lambda tmpdir: tmpdir  # no artifact bucket here
    mod = types.ModuleType("antenv.axon_hooks")
    mod.get_axon_ntff_profile_hook = lambda: hook
    mod.set_axon_ntff_profile_hook = lambda h: None
    sys.modules["antenv.axon_hooks"] = mod


def _build():
    nc = bacc.Bacc(None)

    xqT = nc.declare_dram_parameter("xqT", [D, SEQ], F16, isOutput=False)
    xkT = nc.declare_dram_parameter("xkT", [D, SEQ], F16, isOutput=False)
    xvT = nc.declare_dram_parameter("xvT", [D, SEQ], F16, isOutput=False)
    wqT = nc.declare_dram_parameter("wqT", [D, GD], F16, isOutput=False)
    wkT = nc.declare_dram_parameter("wkT", [D, GD], F16, isOutput=False)
    wvT = nc.declare_dram_parameter("wvT", [D, GD], F16, isOutput=False)
    woT = nc.declare_dram_parameter("woT", [GD, D], F16, isOutput=False)
    maskT = nc.declare_dram_parameter("maskT", [SEQ, SEQ], F16, isOutput=False)
    out = nc.declare_dram_parameter("out", [SEQ, D], F16, isOutput=True)
    debug = bool(int(os.environ.get("KERNEL_DEBUG", "0")))
    if debug:
        dbg = {
            nm: nc.declare_dram_parameter(f"dbg_{nm}", shp, F16, isOutput=True)
            for nm, shp in (
                ("qA", [128, SEQ]), ("kA", [128, SEQ]),
                ("qB", [64, SEQ]), ("kB", [64, SEQ]),
                ("onA", [128, SEQ]), ("onB", [64, SEQ]),
                ("vaug", [128, KT * 3 * (HD + 1)]),
            )
        }

    with tile.TileContext(nc) as tc, ExitStack() as ctx:
        Exp = mybir.ActivationFunctionType.Exp

        # ---- persistent tiles --------------------------------------------
        pp = ctx.enter_context(tc.tile_pool(name="persist", bufs=1))
        qA = pp.tile([128, SEQ], F16, tag="qA")   # heads 0 (p0-63) / 1 (p64-127)
        qB = pp.tile([64, SEQ], F16, tag="qB")    # head 2
        kA = pp.tile([128, SEQ], F16, tag="kA")
        kB = pp.tile([64, SEQ], F16, tag="kB")
        # v in [k-seq, d+1] layout per head; col 64 = ones (softmax denom).
        # Per-head 3D tiles: a 4D [128,KT,3,65] tile sliced [:,m,h,:] loads
        # the PE stationary with misordered columns (observed on HW).
        vaug = [pp.tile([128, KT, HD + 1], F16, tag=f"vaug{h}",
                        name=f"vaug{h}") for h in range(3)]
        onA = pp.tile([128, SEQ], F16, tag="onA")  # normalized out, heads 0/1
        onB = pp.tile([64, SEQ], F16, tag="onB")   # head 2
        woA = pp.tile([128, D], F16, tag="woA")
        woB = pp.tile([64, D], F16, tag="woB")
        w_sb = {n: [pp.tile([128, GD], F16, tag=f"w{n}{k}", name=f"w_{n}_{k}")
                    for k in range(6)] for n in ("q", "k", "v")}
        zt = pp.tile([128, QB], F16, tag="zt")    # zeros for PE warm-up

        nc.vector.memset(zt[:], 0.0)
        for h in range(3):
            nc.vector.memset(vaug[h][:, :, HD : HD + 1], 1.0)

        # weight DMAs on the scalar queue (it is otherwise idle until the
        # first projection PSUM copy ~14us in); wo on sync
        for name, wT in (("q", wqT), ("k", wkT), ("v", wvT)):
            for k in range(6):
                nc.scalar.dma_start(w_sb[name][k][:], wT[ds(k * 128, 128), :])
        nc.sync.dma_start(woA[:], woT[0:128, :])
        nc.sync.dma_start(woB[:], woT[128:GD, :])

        xp = ctx.enter_context(tc.tile_pool(name="xp", bufs=12))
        mp = ctx.enter_context(tc.tile_pool(name="mp", bufs=16))
        # 6 P bufs = 2 full blocks of separation, so a new block's exp never
        # lands in a slot whose attV readers haven't been emitted yet
        pp2 = ctx.enter_context(tc.tile_pool(name="P", bufs=6))
        rp = ctx.enter_context(tc.tile_pool(name="rp", bufs=2))
        op = ctx.enter_context(tc.tile_pool(name="op", bufs=2))

        maskR = maskT.rearrange("(ko ki) q -> ki ko q", ki=128)
        masks = {}

        def issue_mask(n, j0=0, j1=8):
            tiles = masks.setdefault(n, [])
            for j in range(j0, j1):
                t = mp.tile([128, 2, QB], F16, tag="mask", name=f"mask{n}_{j}")
                eng = nc.gpsimd if j % 2 == 0 else nc.sync
                eng.dma_start(t[:], maskR[:, ds(j * 2, 2), ds(n * QB, QB)])
                tiles.append(t)

        # ---- phase 1: projections ----------------------------------------
        with tc.tile_pool(name="pj_ps", bufs=2, space="PSUM") as pj_ps, \
             tc.tile_pool(name="pv_ps", bufs=2, space="PSUM") as pv_ps:

            # PE warm-up: junk matmuls on zeros while the x DMAs stream in.
            # Keeps the HAM clock gate at 8/8 so the first real chains run
            # at 2.4 GHz (~11us of cover).
            wps = pj_ps.tile([128, QB], F32, tag="warm")
            for _ in range(26):
                nc.tensor.matmul(wps[:], lhsT=zt[:, 0:128], rhs=zt[:],
                                 start=True, stop=True)

            dests = {"q": (qA, qB), "k": (kA, kB)}
            for name, xT in (("q", xqT), ("k", xkT), ("v", xvT)):
                for nb2 in range(2):
                    xk = []
                    for k in range(6):
                        xt = xp.tile([128, 1024], F16, tag="x",
                                     name=f"x_{name}_{nb2}_{k}")
                        eng = nc.gpsimd if k % 2 == 0 else nc.sync
                        eng.dma_start(
                            xt[:], xT[ds(k * 128, 128), ds(nb2 * 1024, 1024)]
                        )
                        xk.append(xt)
                    if name == "v":
                        for sb in range(8):
                            kt = nb2 * 8 + sb
                            pv = pv_ps.tile([128, 3, HD], F32, tag="pv")
                            for k in range(6):
                                nc.tensor.matmul(
                                    pv[:, :, :],
                                    lhsT=xk[k][:, ds(sb * 128, 128)],
                                    rhs=w_sb["v"][k][:].rearrange(
                                        "p (h d) -> p h d", h=3
                                    ),
                                    start=(k == 0),
                                    stop=(k == 5),
                                )
                            for h in range(3):
                                nc.vector.tensor_copy(
                                    vaug[h][:, kt, 0:HD], pv[:, h, :]
                                )
                    else:
                        for half in range(2):
                            n = nb2 * 2 + half
                            for mt in range(2):
                                mw = 128 if mt == 0 else 64
                                ps = pj_ps.tile([128, QB], F32, tag="pj")
                                for k in range(6):
                                    nc.tensor.matmul(
                                        ps[0:mw, :],
                                        lhsT=w_sb[name][k][:, ds(mt * 128, mw)],
                                        rhs=xk[k][:, ds(half * QB, QB)],
                                        start=(k == 0),
                                        stop=(k == 5),
                                    )
                                dst = dests[name][mt]
                                nc.scalar.copy(
                                    dst[0:mw, ds(n * QB, QB)], ps[0:mw, :]
                                )
                if name == "q":
                    issue_mask(0)   # early: needed from ~28us on

        issue_mask(1)

        # ---- phase 2: attention + output projection, one pipeline -------
        q_of = (qA, qA, qB)
        k_of = (kA, kA, kB)
        pbase = (0, 64, 0)
        P = {}
        OU = {}
        pending = []   # deferred output-projection chains (n, j)

        with tc.tile_pool(name="e_ps", bufs=2, space="PSUM") as e_ps, \
             tc.tile_pool(name="ou_ps", bufs=3, space="PSUM") as ou_ps, \
             tc.tile_pool(name="f_ps", bufs=1, space="PSUM") as f_ps:

            def energy(n, g, h):
                e = e_ps.tile([128, 2, QB], F32, tag="e")
                p0 = pbase[h]
                for mm in range(2):
                    m = 2 * g + mm
                    nc.tensor.matmul(
                        e[:, mm, :],
                        lhsT=k_of[h][p0 : p0 + 64, ds(m * 128, 128)],
                        rhs=q_of[h][p0 : p0 + 64, ds(n * QB, QB)],
                        start=True,
                        stop=True,
                    )
                sl = ds(2 * g, 2)
                nc.scalar.activation(P[(n, h)][:, sl, :], e[:, :, :], Exp,
                                     scale=SCALE)
                nc.vector.tensor_mul(P[(n, h)][:, sl, :], P[(n, h)][:, sl, :],
                                     masks[n][g][:, :, :])

            def attv_seg(n, g):
                for h in range(3):
                    if g == 0:
                        OU[(n, h)] = ou_ps.tile([HD + 1, QB], F32, tag="ou",
                                                name=f"ou{n}_{h}")
                    ou = OU[(n, h)]
                    for mm in (2 * g, 2 * g + 1):
                        nc.tensor.matmul(
                            ou[:],
                            lhsT=vaug[h][:, mm, :],
                            rhs=P[(n, h)][:, mm, :],
                            start=(mm == 0),
                            stop=(mm == KT - 1),
                        )

            def norm_block(n):
                for h in range(3):
                    ou = OU.pop((n, h))
                    # native tensor_copy remaps partition 64 -> 0; the custom
                    # DVE recip op ignores partition offsets on its operands
                    dsb = rp.tile([1, QB], F32, tag="dsb")
                    nc.vector.tensor_copy(dsb[:], ou[HD : HD + 1, :])
                    r1 = rp.tile([1, QB], F32, tag="r1")
                    nc.vector.reciprocal_approx_fast(r1[:], dsb[:])
                    rb = rp.tile([HD, QB], F32, tag="rb")
                    nc.gpsimd.partition_broadcast(rb[:], r1[:])
                    if h == 2:
                        dst = onB[0:HD, ds(n * QB, QB)]
                    else:
                        dst = onA[pbase[h] : pbase[h] + HD, ds(n * QB, QB)]
                    nc.vector.tensor_mul(dst, ou[0:HD, :], rb[:])

            osb = {}

            def outproj_chain(n, j):
                mq = n * 4 + j // 2
                half = j % 2
                c0 = half * 384
                if half == 0:
                    osb[mq] = op.tile([128, D], F16, tag="o", name=f"o{mq}")
                f = f_ps.tile([128, 384], F32, tag="f")
                nc.tensor.matmul(f[:], lhsT=onA[:, ds(mq * 128, 128)],
                                 rhs=woA[:, ds(c0, 384)], start=True, stop=False)
                nc.tensor.matmul(f[:], lhsT=onB[0:HD, ds(mq * 128, 128)],
                                 rhs=woB[0:HD, ds(c0, 384)], start=False,
                                 stop=True)
                nc.vector.tensor_copy(osb[mq][:, ds(c0, 384)], f[:])
                if half == 1:
                    o = osb.pop(mq)
                    nc.sync.dma_start(out[ds(mq * 128, 128), 0:384],
                                      o[:, 0:384])
                    nc.sync.dma_start(out[ds(mq * 128, 128), 384:D],
                                      o[:, 384:D])

            for s in range(NQB * NG + 1):
                n, g = s // NG, s % NG
                if s < NQB * NG:
                    if g == 0:
                        for h in range(3):
                            P[(n, h)] = pp2.tile([128, KT, QB], F16, tag="P",
                                                 name=f"P{n}_{h}")
                    energy(n, g, 0)
                    energy(n, g, 1)
                if s > 0:
                    attv_seg((s - 1) // NG, (s - 1) % NG)
                if s < NQB * NG:
                    energy(n, g, 2)
                if s > 0 and (s - 1) % NG == NG - 1:
                    norm_block((s - 1) // NG)
                    pending.extend(((s - 1) // NG, j) for j in range(8))
                if pending and s < NQB * NG:
                    outproj_chain(*pending.pop(0))
                # mask prefetch for block n+2, only for groups whose block-n
                # readers (the mask muls) have already been emitted
                if s < NQB * NG and n + 2 < NQB:
                    if g == 5:
                        issue_mask(n + 2, 0, 5)
                    elif g == 7:
                        issue_mask(n + 2, 5, 8)
            while pending:
                outproj_chain(*pending.pop(0))

        if debug:
            for nm, t in (("qA", qA), ("kA", kA), ("qB", qB), ("kB", kB),
                          ("onA", onA), ("onB", onB)):
                nc.sync.dma_start(dbg[nm][0 : t.shape[0], :], t[:])
            nc.sync.dma_start(
                dbg["vaug"][:, 0 : KT * (HD + 1)],
                vaug[0][:].rearrange("p a c -> p (a c)"),
            )

    nc.compile()
    return nc


def kernel(Q, K, V, mask, Wq, Wk, Wv, Wo):
    if "nc" not in _CACHE:
        _CACHE["nc"] = _build()
    nc = _CACHE["nc"]

    maskT_f16 = np.ascontiguousarray((mask[0, 0].T != 0).astype(np.float16))
    in_maps = []
    for c in range(8):
        b, g = c // 4, c % 4
        sl = slice(g * GD, (g + 1) * GD)
        in_maps.append(
            {
                "xqT": np.ascontiguousarray(Q[b].T.astype(np.float16)),
                "xkT": np.ascontiguousarray(K[b].T.astype(np.float16)),
                "xvT": np.ascontiguousarray(V[b].T.astype(np.float16)),
                "wqT": np.ascontiguousarray(Wq[sl, :].T.astype(np.float16)),
                "wkT": np.ascontiguousarray(Wk[sl, :].T.astype(np.float16)),
                "wvT": np.ascontiguousarray(Wv[sl, :].T.astype(np.float16)),
                "woT": np.ascontiguousarray(Wo[:, sl].T.astype(np.float16)),
                "maskT": maskT_f16,
            }
        )

    _install_profile_hook()
    res = run_bass_kernel_spmd(
        nc,
        in_maps,
        core_ids=list(range(8)),
        trace=bool(int(os.environ.get("KERNEL_PROFILE", "0"))),
    )
    _CACHE["last_exec_ns"] = res.exec_time_ns

    out = np.zeros((2, SEQ, D), dtype=np.float32)
    for c in range(8):
        out[c // 4] += res.results[c]["out"].astype(np.float32)
    return out
